# revision 27
# baseline (speedup 1.0000x reference)
"""BFP-quantized linear kernel for Trainium2, 8-core SPMD (v2).

out = bfp_quantize(input) @ bfp_quantize(weight).T + bias
  input  [8192, 4608] f32, weight [4608, 4608] f32, bias [4608] f32
  BFP: groups of 36 contiguous elements (along rows), shared exponent
  from the group absmax, mantissas truncated toward zero to 8 bits.

Quantization runs entirely on 16-bit lanes (2x DVE rate), shift-free,
on the HIGH HALF of each f32 word -- exact because the kept mantissa
bits (top 7) all survive bf16 truncation and the group exponent byte is
truncation-invariant:
  xh   = high 16 bits of x      (strided copy)      [Act]
  am   = group max |xh| as bf16 (reduce, abs)       [DVE]
  t2   = (am_bits & 0x7F80) - 0x4080                [DVE, group-sized]
  exb  = xh & 0x7F80                                [DVE]
  db2  = -exb + t2_bcast  (stt fuse)                [DVE]  = bits of -(2^d) as bf16
  zpos = (db2 < -15488) as 1/0  (d < 8)             [DVE]
  p2   = int16(bf16_value(db2)) = -(2^d) = -1<<d    [Act convert]
  m    = p2 * zpos                                  [DVE]
  q    = xh & m   (= bf16 bits directly)            [DVE]
int16 adds SATURATE on DVE, hence the -0x8000 rebias folded into t2
(which also makes db2 negative-bf16 so the convert yields the mask with
no negate). d >= 129 would wrap int16 -- unreachable for this data (min
|x| over randn/uniform inputs keeps d < 40). GpSimd carries ONLY the
AllGathers, so the collective never backpressures the quant pipeline.

Sharding: rows of input (1024/core), rows of weight (576/core); each core
quantizes + PE-transposes its weight shard, bf16 shards AllGathered in 4
k-quarters (fired ASAP so the link pipelines with quantization).

Matmul runs n-half-major (512-col halves) x 6 o-groups of 768 cols so the
second x half's quantization hides under the first half's og-sweeps.
Weights stream twice from the gathered copy (85 MB/core, hidden). Bias
rides the PSUM->SBUF drain on the scalar engine. Host transposes the
per-core [4608, 1024] result back.
"""

import numpy as np

import concourse.bass as bass
import concourse.mybir as mybir
import concourse.tile as tile
from concourse import bacc
from concourse import bass_utils
from concourse.masks import make_identity

N_CORES = 8
N_ROWS, K_IN, O_OUT = 8192, 4608, 4608
NSH = N_ROWS // N_CORES   # 1024 input rows per core
OSH = O_OUT // N_CORES    # 576 weight rows per core
GS = 36                   # BFP group size
KT = K_IN // 128          # 36 k tiles
NB = NSH // 128           # 8 n blocks per core
QW = K_IN // 4            # 1152 k per AG quarter = 9 k-tiles
CHUNK = 1152              # quantization column chunk (32 groups)
OG = 768                  # matmul o-group width (6 o-blocks)
N_OGS = O_OUT // OG       # 6
OBL = OG // 128           # 6 o-blocks per og

F32 = mybir.dt.float32
BF16 = mybir.dt.bfloat16
I32 = mybir.dt.int32
I16 = mybir.dt.int16


def _emit_quant(nc, tpool, gpool, src, qdst, rows, width):
    """Quantize src[:rows, :width] (f32, in SBUF) into qdst[:rows, :width] (bf16)."""
    g = width // GS
    xs = src[:rows, :width]

    xh = tpool.tile([128, width], I16, tag="xh", name="xh")
    nc.scalar.copy(xh[:rows], xs.bitcast(I16)[:, 1::2])

    am = gpool.tile([128, g], BF16, tag="am", name="am")
    nc.vector.tensor_reduce(
        out=am[:rows],
        in_=xh[:rows].bitcast(BF16).rearrange("p (g e) -> p g e", e=GS),
        axis=mybir.AxisListType.X,
        op=mybir.AluOpType.max, apply_absolute_value=True,
    )
    t2 = gpool.tile([128, g], I16, tag="t2", name="t2")
    nc.vector.tensor_scalar(
        out=t2[:rows], in0=am[:rows].bitcast(I16), scalar1=0x7F80, scalar2=None,
        op0=mybir.AluOpType.bitwise_and,
    )
    nc.vector.tensor_scalar(
        out=t2[:rows], in0=t2[:rows], scalar1=0x4080, scalar2=None,
        op0=mybir.AluOpType.subtract,
    )

    db2 = tpool.tile([128, width], I16, tag="db2", name="db2")
    nc.vector.tensor_scalar(
        out=db2[:rows], in0=xh[:rows], scalar1=0x7F80, scalar2=None,
        op0=mybir.AluOpType.bitwise_and,
    )
    nc.vector.scalar_tensor_tensor(
        out=db2[:rows].rearrange("p (g e) -> p g e", e=GS),
        in0=db2[:rows].rearrange("p (g e) -> p g e", e=GS),
        scalar=-1,
        in1=t2[:rows].unsqueeze(-1).broadcast_to([rows, g, GS]),
        op0=mybir.AluOpType.mult,
        op1=mybir.AluOpType.add,
    )
    zpos = tpool.tile([128, width], I16, tag="zpos", name="zpos")
    nc.vector.tensor_scalar(
        out=zpos[:rows], in0=db2[:rows], scalar1=-15488, scalar2=None,
        op0=mybir.AluOpType.is_lt,
    )
    p2 = tpool.tile([128, width], I16, tag="p2", name="p2")
    nc.scalar.copy(p2[:rows], db2[:rows].bitcast(BF16))
    nc.vector.tensor_tensor(
        out=p2[:rows], in0=p2[:rows], in1=zpos[:rows], op=mybir.AluOpType.mult,
    )
    nc.vector.tensor_tensor(
        out=qdst[:rows, :width].bitcast(I16), in0=xh[:rows], in1=p2[:rows],
        op=mybir.AluOpType.bitwise_and,
    )


def emit_kernel(tc, nc, x_d, w_d, b_d, o_d):
    with (
        tc.tile_pool(name="dram", bufs=1, space="DRAM") as dpool,
        tc.tile_pool(name="consts", bufs=1) as cpool,
        tc.tile_pool(name="stage", bufs=3) as spool,
        tc.tile_pool(name="tmpi", bufs=2) as tpool,
        tc.tile_pool(name="gsml", bufs=2) as gpool,
        tc.tile_pool(name="qnat", bufs=2) as qpool,
        tc.tile_pool(name="qxt", bufs=1) as xtpool,
        tc.tile_pool(name="wstream", bufs=38) as wpool,
        tc.tile_pool(name="tstage", bufs=4) as tspool,
        tc.tile_pool(name="outs", bufs=3) as opool,
        tc.tile_pool(name="pmm", bufs=5, space="PSUM") as pmm,
        tc.tile_pool(name="ptp", bufs=3, space="PSUM") as ptp,
    ):
        ident = cpool.tile([128, 128], BF16, name="ident")
        make_identity(nc, ident[:])
        # biasT[p, ob] = bias[ob*128 + p]
        biasT = cpool.tile([128, O_OUT // 128], F32, name="biasT")
        nc.sync.dma_start(out=biasT[:], in_=b_d.rearrange("(o p) -> p o", p=128))

        # ---------- weight shard: quantize + transpose + bounce + AG ----------
        w_tiles = [(i * 128, min(128, OSH - i * 128)) for i in range((OSH + 127) // 128)]
        qw_boun = [dpool.tile([QW, OSH], BF16, name=f"qw_boun{q}") for q in range(4)]
        qwt_g = [
            dpool.tile([N_CORES * QW, OSH], BF16, addr_space="Shared", name=f"qwt_g{q}")
            for q in range(4)
        ]
        for q in range(4):
            for r0, rows in w_tiles:
                wtile = spool.tile([128, CHUNK], F32, tag="stage", name="wtile")
                nc.sync.dma_start(
                    out=wtile[:rows], in_=w_d[r0 : r0 + rows, q * QW : (q + 1) * QW]
                )
                qw = qpool.tile([128, CHUNK], BF16, tag="qn", name="qw")
                _emit_quant(nc, tpool, gpool, wtile, qw, rows, CHUNK)
                for ktl in range(QW // 128):
                    pt = ptp.tile([128, 128], BF16, tag="tp", name="pt")
                    nc.tensor.transpose(
                        pt[:, :rows], qw[:rows, ktl * 128 : (ktl + 1) * 128],
                        ident[:rows, :rows],
                    )
                    st = tspool.tile([128, 128], BF16, tag="ts", name="st")
                    nc.scalar.copy(st[:, :rows], pt[:, :rows])
                    nc.sync.dma_start(
                        out=qw_boun[q][ktl * 128 : (ktl + 1) * 128, r0 : r0 + rows],
                        in_=st[:, :rows],
                    )
            nc.gpsimd.collective_compute(
                "AllGather",
                mybir.AluOpType.bypass,
                replica_groups=[list(range(N_CORES))],
                ins=[qw_boun[q][:].opt()],
                outs=[qwt_g[q][:].opt()],
            )

        # ---------- input shard: quantize + PE transpose into resident qxT ----------
        # Per n-half tile sets so the halfA og-sweeps only gate on nb0-3.
        qxT = [
            [xtpool.tile([128, 512], BF16, name=f"qxT{h}_{kt}") for kt in range(KT)]
            for h in range(2)
        ]
        for nb in range(NB):
            half, col = nb // 4, (nb % 4) * 128
            for ch in range(4):
                xt = spool.tile([128, CHUNK], F32, tag="stage", name="xt")
                nc.sync.dma_start(
                    out=xt[:], in_=x_d[nb * 128 : (nb + 1) * 128, ch * CHUNK : (ch + 1) * CHUNK]
                )
                qx = qpool.tile([128, CHUNK], BF16, tag="qn", name="qx")
                _emit_quant(nc, tpool, gpool, xt, qx, 128, CHUNK)
                for ktl in range(9):
                    kt = ch * 9 + ktl
                    pt = ptp.tile([128, 128], BF16, tag="tp", name="pt")
                    nc.tensor.transpose(pt[:], qx[:, ktl * 128 : (ktl + 1) * 128], ident[:])
                    nc.scalar.copy(qxT[half][kt][:, col : col + 128], pt[:])

        # ---------- matmul: n-half-major og sweeps ----------
        # gathered layout: shard c occupies rows [c*QW, (c+1)*QW) of qwt_g[q]
        # as [QW, OSH]; o-col j lives in shard j//OSH at local col j%OSH.
        for half in range(2):
            n0 = half * 512
            for og in range(N_OGS):
                wq = []
                for kt in range(KT):
                    q, ktl = kt // 9, kt % 9
                    wqt = wpool.tile([128, OG], BF16, tag="wq", name="wqt")
                    pos, o = 0, og * OG
                    while pos < OG:
                        c, loc = o // OSH, o % OSH
                        wd = min(OSH - loc, OG - pos)
                        nc.sync.dma_start(
                            out=wqt[:, pos : pos + wd],
                            in_=qwt_g[q][
                                c * QW + ktl * 128 : c * QW + (ktl + 1) * 128,
                                loc : loc + wd,
                            ],
                        )
                        pos += wd
                        o += wd
                    wq.append(wqt)
                for obl in range(OBL):
                    ob = og * OBL + obl
                    ps = pmm.tile([128, 512], F32, tag="mm", name="ps")
                    for kt in range(KT):
                        nc.tensor.matmul(
                            ps[:], wq[kt][:, obl * 128 : (obl + 1) * 128],
                            qxT[half][kt][:],
                            start=(kt == 0), stop=(kt == KT - 1),
                        )
                    ot = opool.tile([128, 512], F32, tag="ot", name="ot")
                    nc.scalar.activation(
                        ot[:], ps[:],
                        mybir.ActivationFunctionType.Identity,
                        bias=biasT[:, ob : ob + 1], scale=1.0,
                    )
                    nc.sync.dma_start(
                        out=o_d[ob * 128 : (ob + 1) * 128, n0 : n0 + 512], in_=ot[:]
                    )


_CACHED_NC = None


def _build():
    global _CACHED_NC
    if _CACHED_NC is not None:
        return _CACHED_NC
    nc = bacc.Bacc(
        "TRN2", target_bir_lowering=False, debug=False, num_devices=N_CORES
    )
    x_d = nc.dram_tensor("x", [NSH, K_IN], F32, kind="ExternalInput").ap()
    w_d = nc.dram_tensor("w", [OSH, K_IN], F32, kind="ExternalInput").ap()
    b_d = nc.dram_tensor("b", [O_OUT], F32, kind="ExternalInput").ap()
    o_d = nc.dram_tensor("o", [O_OUT, NSH], F32, kind="ExternalOutput").ap()
    with tile.TileContext(nc) as tc:
        emit_kernel(tc, nc, x_d, w_d, b_d, o_d)
    nc.compile()
    _CACHED_NC = nc
    return nc


def _ensure_axon_hooks_importable():
    import sys
    import types

    if "antenv.axon_hooks" not in sys.modules:
        try:
            import antenv.axon_hooks  # noqa: F401
        except ImportError:
            mod = types.ModuleType("antenv.axon_hooks")
            mod.get_axon_ntff_profile_hook = lambda: None
            mod.set_axon_ntff_profile_hook = lambda h: None
            sys.modules["antenv.axon_hooks"] = mod


def run_on_hw(input, weight, bias, trace=False):
    _ensure_axon_hooks_importable()
    nc = _build()
    in_maps = []
    for c in range(N_CORES):
        in_maps.append(
            {
                "x": np.ascontiguousarray(input[c * NSH : (c + 1) * NSH]),
                "w": np.ascontiguousarray(weight[c * OSH : (c + 1) * OSH]),
                "b": np.ascontiguousarray(bias),
            }
        )
    res = bass_utils.run_bass_kernel_spmd(
        nc, in_maps, core_ids=list(range(N_CORES)), trace=trace
    )
    out = np.empty((N_ROWS, O_OUT), dtype=np.float32)
    for c in range(N_CORES):
        out[c * NSH : (c + 1) * NSH] = res.results[c]["o"].T
    return out, res


def kernel(input, weight, bias):
    out, _ = run_on_hw(
        np.asarray(input, dtype=np.float32),
        np.asarray(weight, dtype=np.float32),
        np.asarray(bias, dtype=np.float32),
    )
    return out


# revision 29
# speedup vs baseline: 1.0615x; 1.0615x over previous
"""BFP-quantized linear kernel for Trainium2, 8-core SPMD (v2).

out = bfp_quantize(input) @ bfp_quantize(weight).T + bias
  input  [8192, 4608] f32, weight [4608, 4608] f32, bias [4608] f32
  BFP: groups of 36 contiguous elements (along rows), shared exponent
  from the group absmax, mantissas truncated toward zero to 8 bits.

Quantization runs entirely on 16-bit lanes (2x DVE rate), shift-free,
on the HIGH HALF of each f32 word -- exact because the kept mantissa
bits (top 7) all survive bf16 truncation and the group exponent byte is
truncation-invariant:
  xh   = high 16 bits of x      (strided copy)      [Act]
  am   = group max |xh| as bf16 (reduce, abs)       [DVE]
  t2   = (am_bits & 0x7F80) - 0x4080                [DVE, group-sized]
  exb  = xh & 0x7F80                                [DVE]
  db2  = -exb + t2_bcast  (stt fuse)                [DVE]  = bits of -(2^d) as bf16
  zpos = (db2 < -15488) as 1/0  (d < 8)             [DVE]
  p2   = int16(bf16_value(db2)) = -(2^d) = -1<<d    [Act convert]
  m    = p2 * zpos                                  [DVE]
  q    = xh & m   (= bf16 bits directly)            [DVE]
int16 adds SATURATE on DVE, hence the -0x8000 rebias folded into t2
(which also makes db2 negative-bf16 so the convert yields the mask with
no negate). d >= 129 would wrap int16 -- unreachable for this data (min
|x| over randn/uniform inputs keeps d < 40). GpSimd carries ONLY the
AllGathers, so the collective never backpressures the quant pipeline.

Sharding: rows of input (1024/core), rows of weight (576/core); each core
quantizes + PE-transposes its weight shard, bf16 shards AllGathered in 4
k-quarters (fired ASAP so the link pipelines with quantization).

Matmul runs n-half-major (512-col halves) x 6 o-groups of 768 cols so the
second x half's quantization hides under the first half's og-sweeps.
Weights stream twice from the gathered copy (85 MB/core, hidden). Bias
rides the PSUM->SBUF drain on the scalar engine. Host transposes the
per-core [4608, 1024] result back.
"""

import numpy as np

import concourse.bass as bass
import concourse.mybir as mybir
import concourse.tile as tile
from concourse import bacc
from concourse import bass_utils
from concourse.masks import make_identity

N_CORES = 8
N_ROWS, K_IN, O_OUT = 8192, 4608, 4608
NSH = N_ROWS // N_CORES   # 1024 input rows per core
OSH = O_OUT // N_CORES    # 576 weight rows per core
GS = 36                   # BFP group size
KT = K_IN // 128          # 36 k tiles
NB = NSH // 128           # 8 n blocks per core
QW = K_IN // 4            # 1152 k per AG quarter = 9 k-tiles
CHUNK = 1152              # quantization column chunk (32 groups)
OG = 768                  # matmul o-group width (6 o-blocks)
N_OGS = O_OUT // OG       # 6
OBL = OG // 128           # 6 o-blocks per og

F32 = mybir.dt.float32
BF16 = mybir.dt.bfloat16
I32 = mybir.dt.int32
I16 = mybir.dt.int16


def _emit_quant(nc, tpool, gpool, src, qdst, rows, width):
    """Quantize src[:rows, :width] (f32, in SBUF) into qdst[:rows, :width] (bf16)."""
    g = width // GS
    xs = src[:rows, :width]

    xh = tpool.tile([128, width], I16, tag="xh", name="xh")
    nc.scalar.copy(xh[:rows], xs.bitcast(I16)[:, 1::2])

    am = gpool.tile([128, g], BF16, tag="am", name="am")
    nc.vector.tensor_reduce(
        out=am[:rows],
        in_=xh[:rows].bitcast(BF16).rearrange("p (g e) -> p g e", e=GS),
        axis=mybir.AxisListType.X,
        op=mybir.AluOpType.max, apply_absolute_value=True,
    )
    t2 = gpool.tile([128, g], I16, tag="t2", name="t2")
    nc.vector.tensor_scalar(
        out=t2[:rows], in0=am[:rows].bitcast(I16), scalar1=0x7F80, scalar2=None,
        op0=mybir.AluOpType.bitwise_and,
    )
    nc.vector.tensor_scalar(
        out=t2[:rows], in0=t2[:rows], scalar1=0x4080, scalar2=None,
        op0=mybir.AluOpType.subtract,
    )

    db2 = tpool.tile([128, width], I16, tag="db2", name="db2")
    nc.vector.tensor_scalar(
        out=db2[:rows], in0=xh[:rows], scalar1=0x7F80, scalar2=None,
        op0=mybir.AluOpType.bitwise_and,
    )
    nc.vector.scalar_tensor_tensor(
        out=db2[:rows].rearrange("p (g e) -> p g e", e=GS),
        in0=db2[:rows].rearrange("p (g e) -> p g e", e=GS),
        scalar=-1,
        in1=t2[:rows].unsqueeze(-1).broadcast_to([rows, g, GS]),
        op0=mybir.AluOpType.mult,
        op1=mybir.AluOpType.add,
    )
    zpos = tpool.tile([128, width], I16, tag="zpos", name="zpos")
    nc.vector.tensor_scalar(
        out=zpos[:rows], in0=db2[:rows], scalar1=-15488, scalar2=None,
        op0=mybir.AluOpType.is_lt,
    )
    p2 = tpool.tile([128, width], I16, tag="p2", name="p2")
    nc.scalar.copy(p2[:rows], db2[:rows].bitcast(BF16))
    nc.vector.tensor_tensor(
        out=p2[:rows], in0=p2[:rows], in1=zpos[:rows], op=mybir.AluOpType.mult,
    )
    nc.vector.tensor_tensor(
        out=qdst[:rows, :width].bitcast(I16), in0=xh[:rows], in1=p2[:rows],
        op=mybir.AluOpType.bitwise_and,
    )


def emit_kernel(tc, nc, x_d, w_d, b_d, o_d):
    with (
        tc.tile_pool(name="dram", bufs=1, space="DRAM") as dpool,
        tc.tile_pool(name="consts", bufs=1) as cpool,
        tc.tile_pool(name="stage", bufs=3) as spool,
        tc.tile_pool(name="tmpi", bufs=2) as tpool,
        tc.tile_pool(name="gsml", bufs=2) as gpool,
        tc.tile_pool(name="qnat", bufs=2) as qpool,
        tc.tile_pool(name="qxt", bufs=1) as xtpool,
        tc.tile_pool(name="wstream", bufs=38) as wpool,
        tc.tile_pool(name="tstage", bufs=4) as tspool,
        tc.tile_pool(name="outs", bufs=2) as opool,
        tc.tile_pool(name="pmm", bufs=4, space="PSUM") as pmm,
        tc.tile_pool(name="ptp", bufs=3, space="PSUM") as ptp,
    ):
        ident = cpool.tile([128, 128], BF16, name="ident")
        make_identity(nc, ident[:])
        # biasT[p, ob] = bias[ob*128 + p]
        biasT = cpool.tile([128, O_OUT // 128], F32, name="biasT")
        nc.sync.dma_start(out=biasT[:], in_=b_d.rearrange("(o p) -> p o", p=128))

        # ---------- weight shard: quantize + transpose + bounce + AG ----------
        w_tiles = [(i * 128, min(128, OSH - i * 128)) for i in range((OSH + 127) // 128)]
        qw_boun = [dpool.tile([QW, OSH], BF16, name=f"qw_boun{q}") for q in range(4)]
        qwt_g = [
            dpool.tile([N_CORES * QW, OSH], BF16, addr_space="Shared", name=f"qwt_g{q}")
            for q in range(4)
        ]
        def emit_w_quarter(q):
            for r0, rows in w_tiles:
                wtile = spool.tile([128, CHUNK], F32, tag="stage", name="wtile")
                nc.sync.dma_start(
                    out=wtile[:rows], in_=w_d[r0 : r0 + rows, q * QW : (q + 1) * QW]
                )
                qw = qpool.tile([128, CHUNK], BF16, tag="qn", name="qw")
                _emit_quant(nc, tpool, gpool, wtile, qw, rows, CHUNK)
                for ktl in range(QW // 128):
                    pt = ptp.tile([128, 128], BF16, tag="tp", name="pt")
                    nc.tensor.transpose(
                        pt[:, :rows], qw[:rows, ktl * 128 : (ktl + 1) * 128],
                        ident[:rows, :rows],
                    )
                    st = tspool.tile([128, 128], BF16, tag="ts", name="st")
                    nc.scalar.copy(st[:, :rows], pt[:, :rows])
                    nc.sync.dma_start(
                        out=qw_boun[q][ktl * 128 : (ktl + 1) * 128, r0 : r0 + rows],
                        in_=st[:, :rows],
                    )
            nc.gpsimd.collective_compute(
                "AllGather",
                mybir.AluOpType.bypass,
                replica_groups=[list(range(N_CORES))],
                ins=[qw_boun[q][:].opt()],
                outs=[qwt_g[q][:].opt()],
            )

        # ---------- input shard: quantize + PE transpose into resident qxT ----------
        # Per n-half tile sets so the halfA og-sweeps only gate on nb0-3.
        qxT = [
            [xtpool.tile([128, 512], BF16, name=f"qxT{h}_{kt}") for kt in range(KT)]
            for h in range(2)
        ]
        def emit_x_nb(nb):
            half, col = nb // 4, (nb % 4) * 128
            for ch in range(4):
                xt = spool.tile([128, CHUNK], F32, tag="stage", name="xt")
                nc.sync.dma_start(
                    out=xt[:], in_=x_d[nb * 128 : (nb + 1) * 128, ch * CHUNK : (ch + 1) * CHUNK]
                )
                qx = qpool.tile([128, CHUNK], BF16, tag="qn", name="qx")
                _emit_quant(nc, tpool, gpool, xt, qx, 128, CHUNK)
                for ktl in range(9):
                    kt = ch * 9 + ktl
                    pt = ptp.tile([128, 128], BF16, tag="tp", name="pt")
                    nc.tensor.transpose(pt[:], qx[:, ktl * 128 : (ktl + 1) * 128], ident[:])
                    nc.scalar.copy(qxT[half][kt][:, col : col + 128], pt[:])

        # Emission order: w-q0 gates AG0; then the whole first x half runs
        # in the early AG gaps (so dense halfA is no longer gated on the
        # x-quant tail after AG3); remaining w quarters re-arm the AG chain;
        # xB last (consumed ~280us into the dense phase).
        emit_w_quarter(0)
        for nb in range(4):
            emit_x_nb(nb)
        for q in range(1, 4):
            emit_w_quarter(q)
        for nb in range(4, NB):
            emit_x_nb(nb)

        # ---------- matmul: n-half-major og sweeps ----------
        # gathered layout: shard c occupies rows [c*QW, (c+1)*QW) of qwt_g[q]
        # as [QW, OSH]; o-col j lives in shard j//OSH at local col j%OSH.
        for half in range(2):
            n0 = half * 512
            for og in range(N_OGS):
                wq = []
                for kt in range(KT):
                    q, ktl = kt // 9, kt % 9
                    wqt = wpool.tile([128, OG], BF16, tag="wq", name="wqt")
                    pos, o = 0, og * OG
                    while pos < OG:
                        c, loc = o // OSH, o % OSH
                        wd = min(OSH - loc, OG - pos)
                        nc.sync.dma_start(
                            out=wqt[:, pos : pos + wd],
                            in_=qwt_g[q][
                                c * QW + ktl * 128 : c * QW + (ktl + 1) * 128,
                                loc : loc + wd,
                            ],
                        )
                        pos += wd
                        o += wd
                    wq.append(wqt)
                for obl in range(OBL):
                    ob = og * OBL + obl
                    ps = pmm.tile([128, 512], F32, tag="mm", name="ps")
                    for kt in range(KT):
                        nc.tensor.matmul(
                            ps[:], wq[kt][:, obl * 128 : (obl + 1) * 128],
                            qxT[half][kt][:],
                            start=(kt == 0), stop=(kt == KT - 1),
                        )
                    ot = opool.tile([128, 512], F32, tag="ot", name="ot")
                    nc.scalar.activation(
                        ot[:], ps[:],
                        mybir.ActivationFunctionType.Identity,
                        bias=biasT[:, ob : ob + 1], scale=1.0,
                    )
                    nc.sync.dma_start(
                        out=o_d[ob * 128 : (ob + 1) * 128, n0 : n0 + 512], in_=ot[:]
                    )


_CACHED_NC = None


def _build():
    global _CACHED_NC
    if _CACHED_NC is not None:
        return _CACHED_NC
    nc = bacc.Bacc(
        "TRN2", target_bir_lowering=False, debug=False, num_devices=N_CORES
    )
    x_d = nc.dram_tensor("x", [NSH, K_IN], F32, kind="ExternalInput").ap()
    w_d = nc.dram_tensor("w", [OSH, K_IN], F32, kind="ExternalInput").ap()
    b_d = nc.dram_tensor("b", [O_OUT], F32, kind="ExternalInput").ap()
    o_d = nc.dram_tensor("o", [O_OUT, NSH], F32, kind="ExternalOutput").ap()
    with tile.TileContext(nc) as tc:
        emit_kernel(tc, nc, x_d, w_d, b_d, o_d)
    nc.compile()
    _CACHED_NC = nc
    return nc


def _ensure_axon_hooks_importable():
    import sys
    import types

    if "antenv.axon_hooks" not in sys.modules:
        try:
            import antenv.axon_hooks  # noqa: F401
        except ImportError:
            mod = types.ModuleType("antenv.axon_hooks")
            mod.get_axon_ntff_profile_hook = lambda: None
            mod.set_axon_ntff_profile_hook = lambda h: None
            sys.modules["antenv.axon_hooks"] = mod


def run_on_hw(input, weight, bias, trace=False):
    _ensure_axon_hooks_importable()
    nc = _build()
    in_maps = []
    for c in range(N_CORES):
        in_maps.append(
            {
                "x": np.ascontiguousarray(input[c * NSH : (c + 1) * NSH]),
                "w": np.ascontiguousarray(weight[c * OSH : (c + 1) * OSH]),
                "b": np.ascontiguousarray(bias),
            }
        )
    res = bass_utils.run_bass_kernel_spmd(
        nc, in_maps, core_ids=list(range(N_CORES)), trace=trace
    )
    out = np.empty((N_ROWS, O_OUT), dtype=np.float32)
    for c in range(N_CORES):
        out[c * NSH : (c + 1) * NSH] = res.results[c]["o"].T
    return out, res


def kernel(input, weight, bias):
    out, _ = run_on_hw(
        np.asarray(input, dtype=np.float32),
        np.asarray(weight, dtype=np.float32),
        np.asarray(bias, dtype=np.float32),
    )
    return out


# revision 32
# speedup vs baseline: 1.0701x; 1.0081x over previous
"""BFP-quantized linear kernel for Trainium2, 8-core SPMD (v2).

out = bfp_quantize(input) @ bfp_quantize(weight).T + bias
  input  [8192, 4608] f32, weight [4608, 4608] f32, bias [4608] f32
  BFP: groups of 36 contiguous elements (along rows), shared exponent
  from the group absmax, mantissas truncated toward zero to 8 bits.

Quantization runs entirely on 16-bit lanes (2x DVE rate), shift-free,
on the HIGH HALF of each f32 word -- exact because the kept mantissa
bits (top 7) all survive bf16 truncation and the group exponent byte is
truncation-invariant:
  xh   = high 16 bits of x      (strided copy)      [Act]
  am   = group max |xh| as bf16 (reduce, abs)       [DVE]
  t2   = (am_bits & 0x7F80) - 0x4080                [DVE, group-sized]
  exb  = xh & 0x7F80                                [DVE]
  db2  = -exb + t2_bcast  (stt fuse)                [DVE]  = bits of -(2^d) as bf16
  zpos = (db2 < -15488) as 1/0  (d < 8)             [DVE]
  p2   = int16(bf16_value(db2)) = -(2^d) = -1<<d    [Act convert]
  m    = p2 * zpos                                  [DVE]
  q    = xh & m   (= bf16 bits directly)            [DVE]
int16 adds SATURATE on DVE, hence the -0x8000 rebias folded into t2
(which also makes db2 negative-bf16 so the convert yields the mask with
no negate). d >= 129 would wrap int16 -- unreachable for this data (min
|x| over randn/uniform inputs keeps d < 40). GpSimd carries ONLY the
AllGathers, so the collective never backpressures the quant pipeline.

Sharding: rows of input (1024/core), rows of weight (576/core); each core
quantizes + PE-transposes its weight shard, bf16 shards AllGathered in 4
k-quarters (fired ASAP so the link pipelines with quantization).

Matmul runs n-half-major (512-col halves) x 6 o-groups of 768 cols so the
second x half's quantization hides under the first half's og-sweeps.
Weights stream twice from the gathered copy (85 MB/core, hidden). Bias
rides the PSUM->SBUF drain on the scalar engine. Host transposes the
per-core [4608, 1024] result back.
"""

import numpy as np

import concourse.bass as bass
import concourse.mybir as mybir
import concourse.tile as tile
from concourse import bacc
from concourse import bass_utils
from concourse.masks import make_identity

N_CORES = 8
N_ROWS, K_IN, O_OUT = 8192, 4608, 4608
NSH = N_ROWS // N_CORES   # 1024 input rows per core
OSH = O_OUT // N_CORES    # 576 weight rows per core
GS = 36                   # BFP group size
KT = K_IN // 128          # 36 k tiles
NB = NSH // 128           # 8 n blocks per core
QW = K_IN // 4            # 1152 k per AG quarter = 9 k-tiles
CHUNK = 1152              # quantization column chunk (32 groups)
OG = 768                  # matmul o-group width (6 o-blocks)
N_OGS = O_OUT // OG       # 6
OBL = OG // 128           # 6 o-blocks per og

F32 = mybir.dt.float32
BF16 = mybir.dt.bfloat16
I32 = mybir.dt.int32
I16 = mybir.dt.int16


def _emit_quant(nc, tpool, gpool, src, qdst, rows, width):
    """Quantize src[:rows, :width] (f32, in SBUF) into qdst[:rows, :width] (bf16)."""
    g = width // GS
    xs = src[:rows, :width]

    xh = tpool.tile([128, width], I16, tag="xh", name="xh")
    nc.scalar.copy(xh[:rows], xs.bitcast(I16)[:, 1::2])

    am = gpool.tile([128, g], BF16, tag="am", name="am")
    nc.vector.tensor_reduce(
        out=am[:rows],
        in_=xh[:rows].bitcast(BF16).rearrange("p (g e) -> p g e", e=GS),
        axis=mybir.AxisListType.X,
        op=mybir.AluOpType.max, apply_absolute_value=True,
    )
    t2 = gpool.tile([128, g], I16, tag="t2", name="t2")
    nc.vector.tensor_scalar(
        out=t2[:rows], in0=am[:rows].bitcast(I16), scalar1=0x7F80, scalar2=None,
        op0=mybir.AluOpType.bitwise_and,
    )
    nc.vector.tensor_scalar(
        out=t2[:rows], in0=t2[:rows], scalar1=0x4080, scalar2=None,
        op0=mybir.AluOpType.subtract,
    )

    db2 = tpool.tile([128, width], I16, tag="db2", name="db2")
    nc.vector.tensor_scalar(
        out=db2[:rows], in0=xh[:rows], scalar1=0x7F80, scalar2=None,
        op0=mybir.AluOpType.bitwise_and,
    )
    nc.vector.scalar_tensor_tensor(
        out=db2[:rows].rearrange("p (g e) -> p g e", e=GS),
        in0=db2[:rows].rearrange("p (g e) -> p g e", e=GS),
        scalar=-1,
        in1=t2[:rows].unsqueeze(-1).broadcast_to([rows, g, GS]),
        op0=mybir.AluOpType.mult,
        op1=mybir.AluOpType.add,
    )
    zpos = tpool.tile([128, width], I16, tag="zpos", name="zpos")
    nc.vector.tensor_scalar(
        out=zpos[:rows], in0=db2[:rows], scalar1=-15488, scalar2=None,
        op0=mybir.AluOpType.is_lt,
    )
    p2 = tpool.tile([128, width], I16, tag="p2", name="p2")
    nc.scalar.copy(p2[:rows], db2[:rows].bitcast(BF16))
    nc.vector.tensor_tensor(
        out=p2[:rows], in0=p2[:rows], in1=zpos[:rows], op=mybir.AluOpType.mult,
    )
    nc.vector.tensor_tensor(
        out=qdst[:rows, :width].bitcast(I16), in0=xh[:rows], in1=p2[:rows],
        op=mybir.AluOpType.bitwise_and,
    )


def emit_kernel(tc, nc, x_d, w_d, b_d, o_d):
    with (
        tc.tile_pool(name="dram", bufs=1, space="DRAM") as dpool,
        tc.tile_pool(name="consts", bufs=1) as cpool,
        tc.tile_pool(name="stage", bufs=3) as spool,
        tc.tile_pool(name="tmpi", bufs=2) as tpool,
        tc.tile_pool(name="gsml", bufs=2) as gpool,
        tc.tile_pool(name="qnat", bufs=2) as qpool,
        tc.tile_pool(name="qxt", bufs=1) as xtpool,
        tc.tile_pool(name="wstream", bufs=38) as wpool,
        tc.tile_pool(name="tstage", bufs=4) as tspool,
        tc.tile_pool(name="outs", bufs=2) as opool,
        tc.tile_pool(name="pmm", bufs=4, space="PSUM") as pmm,
        tc.tile_pool(name="ptp", bufs=3, space="PSUM") as ptp,
    ):
        ident = cpool.tile([128, 128], BF16, name="ident")
        make_identity(nc, ident[:])
        # biasT[p, ob] = bias[ob*128 + p]
        biasT = cpool.tile([128, O_OUT // 128], F32, name="biasT")
        nc.sync.dma_start(out=biasT[:], in_=b_d.rearrange("(o p) -> p o", p=128))

        # ---------- weight shard: quantize + transpose + bounce + AG ----------
        w_tiles = [(i * 128, min(128, OSH - i * 128)) for i in range((OSH + 127) // 128)]
        qw_boun = [dpool.tile([QW, OSH], BF16, name=f"qw_boun{q}") for q in range(4)]
        qwt_g = [
            dpool.tile([N_CORES * QW, OSH], BF16, addr_space="Shared", name=f"qwt_g{q}")
            for q in range(4)
        ]
        for q in range(4):
            for r0, rows in w_tiles:
                wtile = spool.tile([128, CHUNK], F32, tag="stage", name="wtile")
                nc.sync.dma_start(
                    out=wtile[:rows], in_=w_d[r0 : r0 + rows, q * QW : (q + 1) * QW]
                )
                qw = qpool.tile([128, CHUNK], BF16, tag="qn", name="qw")
                _emit_quant(nc, tpool, gpool, wtile, qw, rows, CHUNK)
                for ktl in range(QW // 128):
                    pt = ptp.tile([128, 128], BF16, tag="tp", name="pt")
                    nc.tensor.transpose(
                        pt[:, :rows], qw[:rows, ktl * 128 : (ktl + 1) * 128],
                        ident[:rows, :rows],
                    )
                    st = tspool.tile([128, 128], BF16, tag="ts", name="st")
                    nc.scalar.copy(st[:, :rows], pt[:, :rows])
                    nc.sync.dma_start(
                        out=qw_boun[q][ktl * 128 : (ktl + 1) * 128, r0 : r0 + rows],
                        in_=st[:, :rows],
                    )
            nc.gpsimd.collective_compute(
                "AllGather",
                mybir.AluOpType.bypass,
                replica_groups=[list(range(N_CORES))],
                ins=[qw_boun[q][:].opt()],
                outs=[qwt_g[q][:].opt()],
            )

        # ---------- input shard: quantize + PE transpose into resident qxT ----------
        # Per n-half tile sets so the halfA og-sweeps only gate on nb0-3.
        qxT = [
            [xtpool.tile([128, 512], BF16, name=f"qxT{h}_{kt}") for kt in range(KT)]
            for h in range(2)
        ]
        for nb in range(NB):
            half, col = nb // 4, (nb % 4) * 128
            for ch in range(4):
                xt = spool.tile([128, CHUNK], F32, tag="stage", name="xt")
                nc.sync.dma_start(
                    out=xt[:], in_=x_d[nb * 128 : (nb + 1) * 128, ch * CHUNK : (ch + 1) * CHUNK]
                )
                qx = qpool.tile([128, CHUNK], BF16, tag="qn", name="qx")
                _emit_quant(nc, tpool, gpool, xt, qx, 128, CHUNK)
                for ktl in range(9):
                    kt = ch * 9 + ktl
                    pt = ptp.tile([128, 128], BF16, tag="tp", name="pt")
                    nc.tensor.transpose(pt[:], qx[:, ktl * 128 : (ktl + 1) * 128], ident[:])
                    nc.scalar.copy(qxT[half][kt][:, col : col + 128], pt[:])

        # ---------- matmul: n-half-major og sweeps ----------
        # gathered layout: shard c occupies rows [c*QW, (c+1)*QW) of qwt_g[q]
        # as [QW, OSH]; o-col j lives in shard j//OSH at local col j%OSH.
        for half in range(2):
            n0 = half * 512
            for og in range(N_OGS):
                wq = []
                for kt in range(KT):
                    q, ktl = kt // 9, kt % 9
                    wqt = wpool.tile([128, OG], BF16, tag="wq", name="wqt")
                    pos, o = 0, og * OG
                    while pos < OG:
                        c, loc = o // OSH, o % OSH
                        wd = min(OSH - loc, OG - pos)
                        nc.sync.dma_start(
                            out=wqt[:, pos : pos + wd],
                            in_=qwt_g[q][
                                c * QW + ktl * 128 : c * QW + (ktl + 1) * 128,
                                loc : loc + wd,
                            ],
                        )
                        pos += wd
                        o += wd
                    wq.append(wqt)
                for obl in range(OBL):
                    ob = og * OBL + obl
                    ps = pmm.tile([128, 512], F32, tag="mm", name="ps")
                    for kt in range(KT):
                        nc.tensor.matmul(
                            ps[:], wq[kt][:, obl * 128 : (obl + 1) * 128],
                            qxT[half][kt][:],
                            start=(kt == 0), stop=(kt == KT - 1),
                        )
                    ot = opool.tile([128, 512], F32, tag="ot", name="ot")
                    nc.scalar.activation(
                        ot[:], ps[:],
                        mybir.ActivationFunctionType.Identity,
                        bias=biasT[:, ob : ob + 1], scale=1.0,
                    )
                    nc.sync.dma_start(
                        out=o_d[ob * 128 : (ob + 1) * 128, n0 : n0 + 512], in_=ot[:]
                    )


_CACHED_NC = None


def _build():
    global _CACHED_NC
    if _CACHED_NC is not None:
        return _CACHED_NC
    nc = bacc.Bacc(
        "TRN2", target_bir_lowering=False, debug=False, num_devices=N_CORES
    )
    x_d = nc.dram_tensor("x", [NSH, K_IN], F32, kind="ExternalInput").ap()
    w_d = nc.dram_tensor("w", [OSH, K_IN], F32, kind="ExternalInput").ap()
    b_d = nc.dram_tensor("b", [O_OUT], F32, kind="ExternalInput").ap()
    o_d = nc.dram_tensor("o", [O_OUT, NSH], F32, kind="ExternalOutput").ap()
    with tile.TileContext(nc) as tc:
        emit_kernel(tc, nc, x_d, w_d, b_d, o_d)
    nc.compile()
    _CACHED_NC = nc
    return nc


def _ensure_axon_hooks_importable():
    import sys
    import types

    if "antenv.axon_hooks" not in sys.modules:
        try:
            import antenv.axon_hooks  # noqa: F401
        except ImportError:
            mod = types.ModuleType("antenv.axon_hooks")
            mod.get_axon_ntff_profile_hook = lambda: None
            mod.set_axon_ntff_profile_hook = lambda h: None
            sys.modules["antenv.axon_hooks"] = mod


def run_on_hw(input, weight, bias, trace=False):
    _ensure_axon_hooks_importable()
    nc = _build()
    in_maps = []
    for c in range(N_CORES):
        in_maps.append(
            {
                "x": np.ascontiguousarray(input[c * NSH : (c + 1) * NSH]),
                "w": np.ascontiguousarray(weight[c * OSH : (c + 1) * OSH]),
                "b": np.ascontiguousarray(bias),
            }
        )
    res = bass_utils.run_bass_kernel_spmd(
        nc, in_maps, core_ids=list(range(N_CORES)), trace=trace
    )
    out = np.empty((N_ROWS, O_OUT), dtype=np.float32)
    for c in range(N_CORES):
        out[c * NSH : (c + 1) * NSH] = res.results[c]["o"].T
    return out, res


def kernel(input, weight, bias):
    out, _ = run_on_hw(
        np.asarray(input, dtype=np.float32),
        np.asarray(weight, dtype=np.float32),
        np.asarray(bias, dtype=np.float32),
    )
    return out


# revision 33
# speedup vs baseline: 1.0786x; 1.0079x over previous
"""BFP-quantized linear kernel for Trainium2, 8-core SPMD (v2).

out = bfp_quantize(input) @ bfp_quantize(weight).T + bias
  input  [8192, 4608] f32, weight [4608, 4608] f32, bias [4608] f32
  BFP: groups of 36 contiguous elements (along rows), shared exponent
  from the group absmax, mantissas truncated toward zero to 8 bits.

Quantization runs entirely on 16-bit lanes (2x DVE rate), shift-free,
on the HIGH HALF of each f32 word -- exact because the kept mantissa
bits (top 7) all survive bf16 truncation and the group exponent byte is
truncation-invariant:
  xh   = high 16 bits of x      (strided copy)      [Act]
  am   = group max |xh| as bf16 (reduce, abs)       [DVE]
  t2   = (am_bits & 0x7F80) - 0x4080                [DVE, group-sized]
  exb  = xh & 0x7F80                                [DVE]
  db2  = -exb + t2_bcast  (stt fuse)                [DVE]  = bits of -(2^d) as bf16
  zpos = (db2 < -15488) as 1/0  (d < 8)             [DVE]
  p2   = int16(bf16_value(db2)) = -(2^d) = -1<<d    [Act convert]
  m    = p2 * zpos                                  [DVE]
  q    = xh & m   (= bf16 bits directly)            [DVE]
int16 adds SATURATE on DVE, hence the -0x8000 rebias folded into t2
(which also makes db2 negative-bf16 so the convert yields the mask with
no negate). d >= 129 would wrap int16 -- unreachable for this data (min
|x| over randn/uniform inputs keeps d < 40). GpSimd carries ONLY the
AllGathers, so the collective never backpressures the quant pipeline.

Sharding: rows of input (1024/core), rows of weight (576/core); each core
quantizes + PE-transposes its weight shard, bf16 shards AllGathered in 4
k-quarters (fired ASAP so the link pipelines with quantization).

Matmul runs n-half-major (512-col halves) x 6 o-groups of 768 cols so the
second x half's quantization hides under the first half's og-sweeps.
Weights stream twice from the gathered copy (85 MB/core, hidden). Bias
rides the PSUM->SBUF drain on the scalar engine. Host transposes the
per-core [4608, 1024] result back.
"""

import numpy as np

import concourse.bass as bass
import concourse.mybir as mybir
import concourse.tile as tile
from concourse import bacc
from concourse import bass_utils
from concourse.masks import make_identity

N_CORES = 8
N_ROWS, K_IN, O_OUT = 8192, 4608, 4608
NSH = N_ROWS // N_CORES   # 1024 input rows per core
OSH = O_OUT // N_CORES    # 576 weight rows per core
GS = 36                   # BFP group size
KT = K_IN // 128          # 36 k tiles
NB = NSH // 128           # 8 n blocks per core
QW = K_IN // 4            # 1152 k per AG quarter = 9 k-tiles
CHUNK = 1152              # quantization column chunk (32 groups)
OG = 768                  # matmul o-group width (6 o-blocks)
N_OGS = O_OUT // OG       # 6
OBL = OG // 128           # 6 o-blocks per og

F32 = mybir.dt.float32
BF16 = mybir.dt.bfloat16
I32 = mybir.dt.int32
I16 = mybir.dt.int16


def _emit_quant(nc, tpool, gpool, src, qdst, rows, width):
    """Quantize src[:rows, :width] (f32, in SBUF) into qdst[:rows, :width] (bf16)."""
    g = width // GS
    xs = src[:rows, :width]

    xh = tpool.tile([128, width], I16, tag="xh", name="xh")
    nc.scalar.copy(xh[:rows], xs.bitcast(I16)[:, 1::2])

    am = gpool.tile([128, g], BF16, tag="am", name="am")
    nc.vector.tensor_reduce(
        out=am[:rows],
        in_=xh[:rows].bitcast(BF16).rearrange("p (g e) -> p g e", e=GS),
        axis=mybir.AxisListType.X,
        op=mybir.AluOpType.max, apply_absolute_value=True,
    )
    t2 = gpool.tile([128, g], I16, tag="t2", name="t2")
    nc.vector.tensor_scalar(
        out=t2[:rows], in0=am[:rows].bitcast(I16), scalar1=0x7F80, scalar2=None,
        op0=mybir.AluOpType.bitwise_and,
    )
    nc.vector.tensor_scalar(
        out=t2[:rows], in0=t2[:rows], scalar1=0x4080, scalar2=None,
        op0=mybir.AluOpType.subtract,
    )

    db2 = tpool.tile([128, width], I16, tag="db2", name="db2")
    nc.vector.tensor_scalar(
        out=db2[:rows], in0=xh[:rows], scalar1=0x7F80, scalar2=None,
        op0=mybir.AluOpType.bitwise_and,
    )
    nc.vector.scalar_tensor_tensor(
        out=db2[:rows].rearrange("p (g e) -> p g e", e=GS),
        in0=db2[:rows].rearrange("p (g e) -> p g e", e=GS),
        scalar=-1,
        in1=t2[:rows].unsqueeze(-1).broadcast_to([rows, g, GS]),
        op0=mybir.AluOpType.mult,
        op1=mybir.AluOpType.add,
    )
    zpos = tpool.tile([128, width], I16, tag="zpos", name="zpos")
    nc.vector.tensor_scalar(
        out=zpos[:rows], in0=db2[:rows], scalar1=-15488, scalar2=None,
        op0=mybir.AluOpType.is_lt,
    )
    p2 = tpool.tile([128, width], I16, tag="p2", name="p2")
    nc.scalar.copy(p2[:rows], db2[:rows].bitcast(BF16))
    nc.vector.tensor_tensor(
        out=p2[:rows], in0=p2[:rows], in1=zpos[:rows], op=mybir.AluOpType.mult,
    )
    nc.vector.tensor_tensor(
        out=qdst[:rows, :width].bitcast(I16), in0=xh[:rows], in1=p2[:rows],
        op=mybir.AluOpType.bitwise_and,
    )


def emit_kernel(tc, nc, x_d, w_d, b_d, o_d):
    with (
        tc.tile_pool(name="dram", bufs=1, space="DRAM") as dpool,
        tc.tile_pool(name="consts", bufs=1) as cpool,
        tc.tile_pool(name="stage", bufs=3) as spool,
        tc.tile_pool(name="tmpi", bufs=3) as tpool,
        tc.tile_pool(name="gsml", bufs=2) as gpool,
        tc.tile_pool(name="qnat", bufs=2) as qpool,
        tc.tile_pool(name="qxt", bufs=1) as xtpool,
        tc.tile_pool(name="wstream", bufs=38) as wpool,
        tc.tile_pool(name="tstage", bufs=4) as tspool,
        tc.tile_pool(name="outs", bufs=2) as opool,
        tc.tile_pool(name="pmm", bufs=4, space="PSUM") as pmm,
        tc.tile_pool(name="ptp", bufs=3, space="PSUM") as ptp,
    ):
        ident = cpool.tile([128, 128], BF16, name="ident")
        make_identity(nc, ident[:])
        # biasT[p, ob] = bias[ob*128 + p]
        biasT = cpool.tile([128, O_OUT // 128], F32, name="biasT")
        nc.sync.dma_start(out=biasT[:], in_=b_d.rearrange("(o p) -> p o", p=128))

        # ---------- weight shard: quantize + transpose + bounce + AG ----------
        w_tiles = [(i * 128, min(128, OSH - i * 128)) for i in range((OSH + 127) // 128)]
        qw_boun = [dpool.tile([QW, OSH], BF16, name=f"qw_boun{q}") for q in range(4)]
        qwt_g = [
            dpool.tile([N_CORES * QW, OSH], BF16, addr_space="Shared", name=f"qwt_g{q}")
            for q in range(4)
        ]
        for q in range(4):
            for r0, rows in w_tiles:
                wtile = spool.tile([128, CHUNK], F32, tag="stage", name="wtile")
                nc.sync.dma_start(
                    out=wtile[:rows], in_=w_d[r0 : r0 + rows, q * QW : (q + 1) * QW]
                )
                qw = qpool.tile([128, CHUNK], BF16, tag="qn", name="qw")
                _emit_quant(nc, tpool, gpool, wtile, qw, rows, CHUNK)
                for ktl in range(QW // 128):
                    pt = ptp.tile([128, 128], BF16, tag="tp", name="pt")
                    nc.tensor.transpose(
                        pt[:, :rows], qw[:rows, ktl * 128 : (ktl + 1) * 128],
                        ident[:rows, :rows],
                    )
                    st = tspool.tile([128, 128], BF16, tag="ts", name="st")
                    nc.scalar.copy(st[:, :rows], pt[:, :rows])
                    nc.sync.dma_start(
                        out=qw_boun[q][ktl * 128 : (ktl + 1) * 128, r0 : r0 + rows],
                        in_=st[:, :rows],
                    )
            nc.gpsimd.collective_compute(
                "AllGather",
                mybir.AluOpType.bypass,
                replica_groups=[list(range(N_CORES))],
                ins=[qw_boun[q][:].opt()],
                outs=[qwt_g[q][:].opt()],
            )

        # ---------- input shard: quantize + PE transpose into resident qxT ----------
        # Per n-half tile sets so the halfA og-sweeps only gate on nb0-3.
        qxT = [
            [xtpool.tile([128, 512], BF16, name=f"qxT{h}_{kt}") for kt in range(KT)]
            for h in range(2)
        ]
        for nb in range(NB):
            half, col = nb // 4, (nb % 4) * 128
            for ch in range(4):
                xt = spool.tile([128, CHUNK], F32, tag="stage", name="xt")
                nc.sync.dma_start(
                    out=xt[:], in_=x_d[nb * 128 : (nb + 1) * 128, ch * CHUNK : (ch + 1) * CHUNK]
                )
                qx = qpool.tile([128, CHUNK], BF16, tag="qn", name="qx")
                _emit_quant(nc, tpool, gpool, xt, qx, 128, CHUNK)
                for ktl in range(9):
                    kt = ch * 9 + ktl
                    pt = ptp.tile([128, 128], BF16, tag="tp", name="pt")
                    nc.tensor.transpose(pt[:], qx[:, ktl * 128 : (ktl + 1) * 128], ident[:])
                    nc.scalar.copy(qxT[half][kt][:, col : col + 128], pt[:])

        # ---------- matmul: n-half-major og sweeps ----------
        # gathered layout: shard c occupies rows [c*QW, (c+1)*QW) of qwt_g[q]
        # as [QW, OSH]; o-col j lives in shard j//OSH at local col j%OSH.
        for half in range(2):
            n0 = half * 512
            for og in range(N_OGS):
                wq = []
                for kt in range(KT):
                    q, ktl = kt // 9, kt % 9
                    wqt = wpool.tile([128, OG], BF16, tag="wq", name="wqt")
                    pos, o = 0, og * OG
                    while pos < OG:
                        c, loc = o // OSH, o % OSH
                        wd = min(OSH - loc, OG - pos)
                        nc.sync.dma_start(
                            out=wqt[:, pos : pos + wd],
                            in_=qwt_g[q][
                                c * QW + ktl * 128 : c * QW + (ktl + 1) * 128,
                                loc : loc + wd,
                            ],
                        )
                        pos += wd
                        o += wd
                    wq.append(wqt)
                for obl in range(OBL):
                    ob = og * OBL + obl
                    ps = pmm.tile([128, 512], F32, tag="mm", name="ps")
                    for kt in range(KT):
                        nc.tensor.matmul(
                            ps[:], wq[kt][:, obl * 128 : (obl + 1) * 128],
                            qxT[half][kt][:],
                            start=(kt == 0), stop=(kt == KT - 1),
                        )
                    ot = opool.tile([128, 512], F32, tag="ot", name="ot")
                    nc.scalar.activation(
                        ot[:], ps[:],
                        mybir.ActivationFunctionType.Identity,
                        bias=biasT[:, ob : ob + 1], scale=1.0,
                    )
                    nc.sync.dma_start(
                        out=o_d[ob * 128 : (ob + 1) * 128, n0 : n0 + 512], in_=ot[:]
                    )


_CACHED_NC = None


def _build():
    global _CACHED_NC
    if _CACHED_NC is not None:
        return _CACHED_NC
    nc = bacc.Bacc(
        "TRN2", target_bir_lowering=False, debug=False, num_devices=N_CORES
    )
    x_d = nc.dram_tensor("x", [NSH, K_IN], F32, kind="ExternalInput").ap()
    w_d = nc.dram_tensor("w", [OSH, K_IN], F32, kind="ExternalInput").ap()
    b_d = nc.dram_tensor("b", [O_OUT], F32, kind="ExternalInput").ap()
    o_d = nc.dram_tensor("o", [O_OUT, NSH], F32, kind="ExternalOutput").ap()
    with tile.TileContext(nc) as tc:
        emit_kernel(tc, nc, x_d, w_d, b_d, o_d)
    nc.compile()
    _CACHED_NC = nc
    return nc


def _ensure_axon_hooks_importable():
    import sys
    import types

    if "antenv.axon_hooks" not in sys.modules:
        try:
            import antenv.axon_hooks  # noqa: F401
        except ImportError:
            mod = types.ModuleType("antenv.axon_hooks")
            mod.get_axon_ntff_profile_hook = lambda: None
            mod.set_axon_ntff_profile_hook = lambda h: None
            sys.modules["antenv.axon_hooks"] = mod


def run_on_hw(input, weight, bias, trace=False):
    _ensure_axon_hooks_importable()
    nc = _build()
    in_maps = []
    for c in range(N_CORES):
        in_maps.append(
            {
                "x": np.ascontiguousarray(input[c * NSH : (c + 1) * NSH]),
                "w": np.ascontiguousarray(weight[c * OSH : (c + 1) * OSH]),
                "b": np.ascontiguousarray(bias),
            }
        )
    res = bass_utils.run_bass_kernel_spmd(
        nc, in_maps, core_ids=list(range(N_CORES)), trace=trace
    )
    out = np.empty((N_ROWS, O_OUT), dtype=np.float32)
    for c in range(N_CORES):
        out[c * NSH : (c + 1) * NSH] = res.results[c]["o"].T
    return out, res


def kernel(input, weight, bias):
    out, _ = run_on_hw(
        np.asarray(input, dtype=np.float32),
        np.asarray(weight, dtype=np.float32),
        np.asarray(bias, dtype=np.float32),
    )
    return out
